# revision 1
# baseline (speedup 1.0000x reference)
"""Trainium2 Bass kernel for nn_BLoss: loss = mean_i(max(0, sum_j B[i,j] - 1)).

Data-parallel over 8 NeuronCores; each core streams a [1024, 16384] f32 row
shard from HBM in [128, W] chunks. Trace-measured facts this design rests on:

- The per-core stream is bound by the 16 SDMA engines' read-side datapath at
  ~26.8 GB/s each (~429 GB/s/core); the HBM stacks behind these tunneled
  cores are not shared, so the SBUF-AXI/engine path is the wall.
- Chunks 1.. stream via SWDGE cast-DMA (nc.gpsimd, f32 -> bf16). The read
  side is the cap either way, but halving SBUF-write bytes relieves the
  2:1-muxed SBUF AXI ports: stream 156.4 us vs 159.8 us for the same 64 MiB.
  bf16 rounding of uniform[0,1) data is unbiased and averages out across
  16384-col rows (measured end-to-end rel err 3.3e-4, tolerance 2e-2).
- Chunk 0 goes through HWDGE (nc.sync) in f32: the Sync engine reaches the
  kernel body first and HWDGE first-byte latency beats the Q7 SWDGE
  emission path, starting the stream ~1 us earlier.
- Per-chunk row sums run on two engines in parallel (VectorE tensor_reduce
  ~1.07 ns/col, ScalarE activation-Copy accum_out ~0.93 ns/col + fixed
  ~280 ns accumulator read), split 5/8 DVE / 3/8 ACT.
- Row tile 7 streams a steeply descending tail (8192, 4096, 2048, 1792, 256)
  with the last chunk DVE-only, so after the last byte only a ~0.4 us reduce
  remains ahead of the fixed ~4.5 us of sem-propagation + output chain.
- Tiles 0-6 rowsums/hinge/partial-sum run mid-stream while tile 7 drains; a
  -1.0 constant column folded into tile 7's partials lets the final hinge +
  combine collapse into one tensor_scalar (MAX 0, ADD hsum06).
- The 128 per-partition hinge sums collapse via a bf16 ones-matmul on
  TensorE (single-pass; fp32 would need a LOW/HIGH pair) so the output DMA
  is a single 4-byte descriptor (a [128,1] output pays ~7 us in straggling
  completion receipts). Host sums the 8 per-core scalars and divides by the
  global batch.

Best measured: 170974 ns (baseline this session: 175322 ns).
"""

import numpy as np
from contextlib import ExitStack

import concourse.bass as bass
import concourse.tile as tile
from concourse import bacc, mybir
from concourse.bass_utils import run_bass_kernel_spmd

N_CORES = 8
ROWS, COLS = 8192, 16384
SHARD_ROWS = ROWS // N_CORES  # 1024
P = 128                       # SBUF partitions
N_RT = SHARD_ROWS // P        # 8 row tiles per core
CHUNK = 8192
PENALTY_B = 1.0

# (row_tile, col_offset, width, dve_cols) per chunk. 5/8 of a big chunk to
# VectorE, the rest to ScalarE accum-copy (A/B-tuned in the v1 session; both
# engines clear a chunk well inside its ~9.7 us DMA arrival window). The tail
# leans DVE-harder as chunks shrink (ScalarE pays a fixed ~280 ns
# accumulator-read per chunk); the final 256-col chunk is DVE-only so the
# post-stream reduce is ~0.4 us.
TAIL = [(8192, 5120), (4096, 2560), (2048, 1280), (1792, 1120), (256, 256)]


def _chunks():
    out = []
    for r in range(N_RT - 1):
        out.append((r, 0, CHUNK, CHUNK * 5 // 8))
        out.append((r, CHUNK, CHUNK, CHUNK * 5 // 8))
    col = 0
    for w, dve in TAIL:
        out.append((N_RT - 1, col, w, dve))
        col += w
    assert col == COLS
    return out


CHUNKS = _chunks()
N_PARTS = sum(2 if dve < w else 1 for (_, _, w, dve) in CHUNKS)

_PROGRAM = None


def _build_program() -> bass.Bass:
    nc = bacc.Bacc("TRN2", target_bir_lowering=False, debug=False)
    B = nc.declare_dram_parameter(
        "B", [SHARD_ROWS, COLS], mybir.dt.float32, isOutput=False
    )
    out = nc.declare_dram_parameter("out", [1, 1], mybir.dt.float32, isOutput=True)

    with ExitStack() as ctx:
        tc = ctx.enter_context(tile.TileContext(nc))
        data = ctx.enter_context(tc.tile_pool(name="data", bufs=8))
        data0 = ctx.enter_context(tc.tile_pool(name="data0", bufs=1))
        stats = ctx.enter_context(tc.tile_pool(name="stats", bufs=1))
        psum = ctx.enter_context(tc.tile_pool(name="psum", bufs=1, space="PSUM"))

        # Partials: per row tile a contiguous column group, so the per-tile
        # rowsum needs no gather. Tiles 0-6 have 4 partials (2 chunks x 2
        # engines); tile 7 has 9 (see TAIL) plus a -1.0 constant column, so
        # the final reduce directly yields rowsum7 - 1 and the hinge+combine
        # collapses into one tensor_scalar (MAX 0, then ADD hsum06).
        n_t7 = sum(2 if dve < w else 1 for (r, _, w, dve) in CHUNKS if r == N_RT - 1)
        n06 = 4 * (N_RT - 1)
        sums = stats.tile([P, n06 + n_t7 + 1], mybir.dt.float32)
        dummy = stats.tile([P, CHUNK * 3 // 8], mybir.dt.bfloat16)
        ones = stats.tile([P, 1], mybir.dt.bfloat16)
        nc.vector.memset(ones[:], 1.0)
        nc.vector.memset(sums[:, n06 + n_t7 :], -1.0)

        pcol = 0
        for i, (r, c0, w, dve_cols) in enumerate(CHUNKS):
            # Chunk 0 goes through HWDGE in f32: the Sync engine reaches the
            # body first and HWDGE's first-byte latency is ~0.5 us shorter
            # than the Q7 SWDGE emission path, so the stream starts earlier.
            # The rest stream through SWDGE as cast f32->bf16.
            if i == 0:
                t = data0.tile([P, w], mybir.dt.float32, tag="t0")
                nc.sync.dma_start(t[:], B[r * P : (r + 1) * P, c0 : c0 + w])
            else:
                t = data.tile([P, w], mybir.dt.bfloat16, tag="t")
                nc.gpsimd.dma_start(t[:], B[r * P : (r + 1) * P, c0 : c0 + w])
            nc.vector.reduce_sum(
                sums[:, pcol : pcol + 1], t[:, :dve_cols], axis=mybir.AxisListType.X
            )
            pcol += 1
            if dve_cols < w:
                nc.scalar.activation(
                    dummy[:, : w - dve_cols],
                    t[:, dve_cols:w],
                    mybir.ActivationFunctionType.Copy,
                    accum_out=sums[:, pcol : pcol + 1],
                )
                pcol += 1
            # Tiles 0-6 stats run mid-stream, right after tile 6's chunks:
            # DVE is otherwise idle while tile 7's tail streams in, so only
            # tile 7's reduce + one fused tensor_scalar remain after the
            # last byte lands.
            if i == 2 * (N_RT - 1) - 1:
                rowsums06 = stats.tile([P, N_RT - 1], mybir.dt.float32)
                nc.vector.reduce_sum(
                    rowsums06[:],
                    sums[:, :n06].rearrange("p (r c) -> p r c", c=4),
                    axis=mybir.AxisListType.X,
                )
                hinge06 = stats.tile([P, N_RT - 1], mybir.dt.float32)
                nc.vector.tensor_scalar(
                    hinge06[:],
                    rowsums06[:],
                    -1.0,
                    0.0,
                    op0=mybir.AluOpType.add,
                    op1=mybir.AluOpType.max,
                )
                hsum06 = stats.tile([P, 1], mybir.dt.float32)
                nc.vector.reduce_sum(
                    hsum06[:], hinge06[:], axis=mybir.AxisListType.X
                )
        assert pcol == n06 + n_t7

        # rowsum7 - 1 (the -1 rides as a constant partial column).
        t7sum = stats.tile([P, 1], mybir.dt.float32)
        nc.vector.reduce_sum(t7sum[:], sums[:, n06:], axis=mybir.AxisListType.X)
        # hsum = max(rowsum7 - 1, 0) + hsum06, in one DVE op. bf16 out ->
        # single-pass PE matmul (fp32 needs a LOW/HIGH LDWEIGHTS+MATMUL
        # pair). ulp(65536)=256 per partition, iid across 128 partitions ->
        # ~1e-4 rel on the final loss. Tolerance is 2e-2.
        hsum = stats.tile([P, 1], mybir.dt.bfloat16)
        with nc.allow_low_precision(reason="bf16 hsum: 256-ulp on 8.4M total, iid across partitions; tolerance 2e-2"):
            nc.vector.tensor_scalar(
                hsum[:],
                t7sum[:],
                0.0,
                hsum06[:, 0:1],
                op0=mybir.AluOpType.max,
                op1=mybir.AluOpType.add,
            )

        acc = psum.tile([1, 1], mybir.dt.float32)
        nc.tensor.matmul(acc[:], ones[:], hsum[:], start=True, stop=True)
        res = stats.tile([1, 1], mybir.dt.float32)
        nc.scalar.copy(res[:], acc[:])
        nc.sync.dma_start(out[:], res[:])

    nc.compile()
    return nc


def _run(B: np.ndarray, trace: bool = False):
    global _PROGRAM
    if _PROGRAM is None:
        _PROGRAM = _build_program()
    in_maps = [
        {"B": B[i * SHARD_ROWS : (i + 1) * SHARD_ROWS]} for i in range(N_CORES)
    ]
    res = run_bass_kernel_spmd(_PROGRAM, in_maps, list(range(N_CORES)), trace=trace)
    total = float(sum(np.float64(r["out"][0, 0]) for r in res.results))
    value = np.asarray(np.float32(PENALTY_B * total / ROWS))
    return value, res


def kernel(B: np.ndarray) -> np.ndarray:
    B = np.ascontiguousarray(np.asarray(B, dtype=np.float32))
    assert B.shape == (ROWS, COLS), B.shape
    value, _ = _run(B, trace=False)
    return value

